# revision 1
# baseline (speedup 1.0000x reference)
"""DOM pooling (segment mean+max over pulses, then linear projection) on 8 trn2 cores.

Strategy:
  Host: bucket DOMs by exact pulse count k ("classes"); deal DOMs of each class
  round-robin across the 8 cores so every core has an identical class structure
  (same per-class DOM count m_k, padded with dummy DOMs). Full 128-DOM windows
  are emitted per class; the leftover (<128) DOMs of every class are packed
  together into shared "ragged" windows (sorted by k, per-DOM slots padded to
  the window capacity by duplicating the DOM's first slot — max-neutral; the
  sum is corrected on device by subtracting padcnt*x0 and scaled by 1/k).
  Each core gets a gathered slot buffer where a DOM's k pulse embeddings are
  stored embed-major (64 x k contiguous), so device reads are sequential.

  Device (one NEFF, SPMD on 8 cores), per 128-DOM window:
    - batched contiguous DMA loads
    - DVE reduce_sum / reduce_max over the slots (contiguous inner axis)
    - PE transpose of [sum|max] concat (128x128) -> PSUM (feat-major)
    - projection matmul out^T = Wk^T.T @ concatT (mean scaling 1/k folded into
      the sum-feature rows of the per-class weights; ragged windows scale on
      DVE and use unscaled weights)
    - ACT adds bias during PSUM->SBUF copy; batched DMA writes out^T.

  Host: scatter per-core transposed outputs back to the full (num_doms, 64).
"""
import sys

import numpy as np

for _p in ("/opt/trn_rl_repo",):
    if _p not in sys.path:
        sys.path.append(_p)

from concourse import bacc
import concourse.mybir as mybir
import concourse.tile as tile
from concourse.bass_utils import run_bass_kernel_spmd
from concourse.masks import make_identity

NCORES = 8
D = 64
FP32 = mybir.dt.float32

last_exec_ns = None  # set when KERNEL_TRACE=1


def _plan(counts):
    """Class/window structure shared by all cores (derived from global counts).

    Returns:
      full_cls: list of (k, fw, col0, base) classes with fw full windows
      rag_cls:  list of (k, r) leftover doms per class (class order)
      rag_win:  list of (k_w, base, col0) ragged windows
      ndcp:     output columns per core
      s_elems:  slot buffer elements per core
    """
    kmax = int(counts.max()) if counts.size else 0
    n_k = np.bincount(counts, minlength=kmax + 1)
    full_cls = []
    rag_cls = []
    col = 0
    slot = 0
    for k in range(1, kmax + 1):
        if n_k[k] == 0:
            continue
        m = -(-int(n_k[k]) // NCORES)
        fw = m // 128
        r = m % 128
        if fw:
            full_cls.append((k, fw, col, slot))
            col += fw * 128
            slot += fw * 128 * k * D
        if r:
            rag_cls.append((k, r))
    # ragged windows: doms in class order (ascending k); capacity = max k in win
    rag_win = []
    R = sum(r for _, r in rag_cls)
    if R:
        ks = np.concatenate([np.full(r, k, np.int32) for k, r in rag_cls])
        RW = -(-R // 128)
        for j in range(RW):
            seg = ks[j * 128 : (j + 1) * 128]
            kw = int(seg.max())
            rag_win.append((kw, slot, col + j * 128))
            slot += 128 * kw * D
        col += RW * 128
    return full_cls, rag_cls, rag_win, col, slot


def _build_nc(full_cls, rag_win, ndcp, s_elems, nwt, nrw):
    nc = bacc.Bacc(None)
    slots_t = nc.dram_tensor("slots", [s_elems], FP32, kind="ExternalInput")
    wts_t = nc.dram_tensor("wts", [nwt * 128, D], FP32, kind="ExternalInput")
    b_t = nc.dram_tensor("b", [D, 1], FP32, kind="ExternalInput")
    if nrw:
        rt_t = nc.dram_tensor("rt", [nrw * 128, 2], FP32, kind="ExternalInput")
    out_t = nc.dram_tensor("out", [D, ndcp], FP32, kind="ExternalOutput")

    with tile.TileContext(nc) as tc:
        with (
            tc.tile_pool(name="const", bufs=1) as constp,
            tc.tile_pool(name="inp", bufs=6) as inp,
            tc.tile_pool(name="mid", bufs=4) as midp,
            tc.tile_pool(name="outp", bufs=4) as outp,
            tc.tile_pool(name="psA", bufs=4, space="PSUM") as psA,
            tc.tile_pool(name="psB", bufs=4, space="PSUM") as psB,
        ):
            ident = constp.tile([128, 128], FP32)
            make_identity(nc, ident[:])
            wt_sb = constp.tile([128, nwt * D], FP32)
            nc.sync.dma_start(
                wt_sb[:].rearrange("p (j e) -> p j e", e=D),
                wts_t[:, :].rearrange("(j p) e -> p j e", p=128),
            )
            b_sb = constp.tile([D, 1], FP32)
            nc.sync.dma_start(b_sb[:], b_t[:])
            if nrw:
                rt_sb = constp.tile([128, nrw * 2], FP32)
                nc.sync.dma_start(
                    rt_sb[:].rearrange("p (j c) -> p j c", c=2),
                    rt_t[:, :].rearrange("(j p) c -> p j c", p=128),
                )

            def window_tail(cat, out_ap, p, jwt):
                """cat: (p, 128) [sum|max] slice; out_ap: (D, p) slice."""
                catT_ps = psA.tile([128, 128], FP32, space="PSUM", tag="ps")
                nc.tensor.transpose(
                    out=catT_ps[:, :p], in_=cat, identity=ident[:p, :p]
                )
                catT = midp.tile([128, 128], FP32, tag="catT")
                nc.scalar.copy(catT[:, :p], catT_ps[:, :p])
                proj_ps = psB.tile([D, 128], FP32, space="PSUM", tag="proj")
                nc.tensor.matmul(
                    proj_ps[:, :p],
                    lhsT=wt_sb[:, jwt * D : (jwt + 1) * D],
                    rhs=catT[:, :p],
                    start=True,
                    stop=True,
                )
                nc.scalar.activation(
                    out_ap, proj_ps[:, :p],
                    mybir.ActivationFunctionType.Identity, bias=b_sb[:, :1],
                )

            # ---- full per-class windows --------------------------------
            for jcls, (k, fw, col0, base) in enumerate(full_cls):
                G = max(1, min(8, 64 // k))
                g = 0
                while g < fw:
                    gw = min(G, fw - g)
                    in_t = inp.tile([128, gw * k * D], FP32, tag="in")
                    src = slots_t[
                        base + g * 128 * k * D : base + (g + gw) * 128 * k * D
                    ].rearrange("(w d f) -> d w f", w=gw, d=128)
                    nc.sync.dma_start(
                        in_t[:].rearrange("d (w f) -> d w f", w=gw), src
                    )
                    cat_g = midp.tile([128, 8 * 128], FP32, tag="cat")
                    co = cat_g[:, : gw * 128].rearrange("d (w c) -> d w c", c=128)
                    if k == 1:
                        v = in_t[:].rearrange("d (w e) -> d w e", w=gw)
                        nc.vector.tensor_copy(co[:, :, 0:D], v)
                        nc.vector.tensor_copy(co[:, :, D:128], v)
                    else:
                        view = in_t[:].rearrange("d (w e s) -> d w e s", w=gw, s=k)
                        nc.vector.reduce_sum(co[:, :, 0:D], view, axis=mybir.AxisListType.X)
                        nc.vector.reduce_max(co[:, :, D:128], view, axis=mybir.AxisListType.X)
                    out_sb = outp.tile([64, 8 * 128], FP32, tag="out")
                    for w in range(gw):
                        window_tail(
                            cat_g[:, w * 128 : (w + 1) * 128],
                            out_sb[:, w * 128 : (w + 1) * 128],
                            128, jcls,
                        )
                    nc.sync.dma_start(
                        out_t[:, col0 + g * 128 : col0 + (g + gw) * 128],
                        out_sb[:, : gw * 128],
                    )
                    g += gw

            # ---- ragged windows (mixed k, capacity k_w) ----------------
            juns = len(full_cls)  # unscaled weight block index
            for j, (kw, base, col0) in enumerate(rag_win):
                in_t = inp.tile([128, kw * D], FP32, tag="in")
                nc.sync.dma_start(
                    in_t[:], slots_t[base : base + 128 * kw * D].rearrange(
                        "(d f) -> d f", d=128
                    ),
                )
                cat_g = midp.tile([128, 8 * 128], FP32, tag="cat")
                view = in_t[:].rearrange("d (e s) -> d e s", s=kw)
                sraw = midp.tile([128, D], FP32, tag="sraw")
                nc.vector.reduce_sum(sraw[:], view, axis=mybir.AxisListType.X)
                nc.vector.reduce_max(cat_g[:, D:128], view, axis=mybir.AxisListType.X)
                # sum correction: (sraw - padcnt*x0) * recip_k
                x0 = view[:, :, 0]
                tmp = midp.tile([128, D], FP32, tag="tmp")
                nc.vector.tensor_scalar_mul(
                    tmp[:], x0, rt_sb[:, j * 2 + 1 : j * 2 + 2]
                )
                nc.vector.tensor_tensor(
                    out=tmp[:], in0=sraw[:], in1=tmp[:], op=mybir.AluOpType.subtract
                )
                nc.vector.tensor_scalar_mul(
                    cat_g[:, 0:D], tmp[:], rt_sb[:, j * 2 : j * 2 + 1]
                )
                out_sb = outp.tile([64, 8 * 128], FP32, tag="out")
                window_tail(cat_g[:, 0:128], out_sb[:, :128], 128, juns)
                nc.sync.dma_start(out_t[:, col0 : col0 + 128], out_sb[:, :128])
    nc.finalize()
    return nc


def kernel(pulse_embeddings, pulse_to_dom_idx, num_doms, proj_w, proj_b):
    global last_exec_ns
    import os

    E = np.ascontiguousarray(np.asarray(pulse_embeddings, dtype=np.float32))
    idx = np.asarray(pulse_to_dom_idx).astype(np.int64)
    nd = int(num_doms)
    W = np.asarray(proj_w, dtype=np.float32)   # (D, 2D)
    b = np.asarray(proj_b, dtype=np.float32)   # (D,)

    counts = np.bincount(idx, minlength=nd)
    full_cls, rag_cls, rag_win, ndcp, s_elems = _plan(counts)
    nwt = len(full_cls) + 1
    nrw = len(rag_win)

    # ---- host-side dom assignment --------------------------------------
    dom_order = np.argsort(counts, kind="stable")
    cs = counts[dom_order]
    n0 = int((counts == 0).sum())
    dom_core = np.full(nd, -1, np.int32)
    dom_col = np.full(nd, -1, np.int32)

    # per-class bookkeeping (shared across cores)
    kmax = int(counts.max()) if counts.size else 0
    n_k = np.bincount(counts, minlength=kmax + 1)
    full_map = {k: (jc, fw, col0, base) for jc, (k, fw, col0, base) in enumerate(full_cls)}
    # ragged: position of each class's leftover run inside the ragged region
    rag_off = {}
    ro = 0
    for k, r in rag_cls:
        rag_off[k] = ro
        ro += r
    R = ro
    rag_col0 = rag_win[0][2] - 0 if rag_win else ndcp  # col of ragged dom 0
    if rag_win:
        rag_col0 = rag_win[0][2]

    off = n0
    # per (class-k, core): number of real doms; and split into full/ragged
    cls_meta = []  # (k, m, n_real, fw, r)
    for k in range(1, kmax + 1):
        if n_k[k] == 0:
            continue
        m = -(-int(n_k[k]) // NCORES)
        fw = m // 128
        r = m % 128
        n_real = int(n_k[k])
        doms_k = dom_order[off : off + n_real]
        off += n_real
        tot = NCORES * m
        core_of = np.arange(tot, dtype=np.int32) % NCORES
        pos_of = np.arange(tot, dtype=np.int32) // NCORES
        # column for position p: in full region if p < fw*128 else ragged
        col_full0 = full_map[k][2] if fw else 0
        p = pos_of[:n_real]
        cols = np.where(
            p < fw * 128,
            col_full0 + p,
            rag_col0 + rag_off.get(k, 0) + (p - fw * 128),
        ).astype(np.int32)
        dom_core[doms_k] = core_of[:n_real]
        dom_col[doms_k] = cols
        cls_meta.append((k, m, n_real, fw, r))

    # pulses sorted by (core, dom column)
    key = dom_core[idx].astype(np.int64) * (1 << 32) + dom_col[idx]
    perm = np.argsort(key, kind="stable")
    core_pulse_counts = np.bincount(dom_core[idx], minlength=NCORES)
    core_splits = np.concatenate([[0], np.cumsum(core_pulse_counts)])

    # ragged window lookup per ragged position
    if nrw:
        rag_kw = np.concatenate(
            [np.full(128, kw, np.int32) for kw, _, _ in rag_win]
        )[: nrw * 128]
        rag_base = np.array([bse for _, bse, _ in rag_win], np.int64)

    # ---- build per-core slot buffers ------------------------------------
    bufs = []
    for c in range(NCORES):
        buf = np.zeros(s_elems, np.float32)
        pc = perm[core_splits[c] : core_splits[c + 1]]
        p_off = 0
        # pass 1: full-window regions, in column order (= ascending k)
        for k, m, n_real, fw, r in cls_meta:
            nreal_c = n_real // NCORES + (1 if c < n_real % NCORES else 0)
            n_full = min(nreal_c, fw * 128)
            if n_full == 0:
                continue
            R_rows = pc[p_off : p_off + n_full * k].reshape(n_full, k)
            p_off += n_full * k
            base = full_map[k][3]
            A = E[R_rows].transpose(0, 2, 1)  # (n, D, k)
            buf[base : base + n_full * D * k] = A.reshape(-1)
        # pass 2: ragged region, in column order (= ascending k)
        for k, m, n_real, fw, r in cls_meta:
            nreal_c = n_real // NCORES + (1 if c < n_real % NCORES else 0)
            n_full = min(nreal_c, fw * 128)
            n_rag = nreal_c - n_full
            if n_rag == 0:
                continue
            R_rows = pc[p_off : p_off + n_rag * k].reshape(n_rag, k)
            p_off += n_rag * k
            rp0 = rag_off[k]
            Arag = E[R_rows].transpose(0, 2, 1)  # (n_rag, D, k)
            i = 0
            while i < n_rag:
                rp = rp0 + i
                j = rp // 128
                kw = int(rag_kw[rp])
                lim = min(n_rag, (j + 1) * 128 - rp0)  # same-window chunk
                chunk = Arag[i:lim]                    # (cn, D, k)
                cn = chunk.shape[0]
                blk = np.empty((cn, D, kw), np.float32)
                blk[:, :, :k] = chunk
                if kw > k:
                    blk[:, :, k:] = chunk[:, :, 0:1]
                bse = int(rag_base[j]) + (rp - j * 128) * D * kw
                buf[bse : bse + cn * D * kw] = blk.reshape(-1)
                i = lim
        bufs.append(buf)

    # ---- weights / tables ----------------------------------------------
    WT = np.ascontiguousarray(W.T)  # (2D, D)
    wts = np.empty((nwt * 128, D), np.float32)
    for jc, (k, fw, col0, base) in enumerate(full_cls):
        blk = WT.copy()
        blk[0:D] *= np.float32(1.0 / k)
        wts[jc * 128 : (jc + 1) * 128] = blk
    wts[len(full_cls) * 128 :] = WT  # unscaled for ragged
    b_col = b.reshape(D, 1)

    rt = None
    if nrw:
        rt = np.zeros((nrw * 128, 2), np.float32)
        rt[:, 0] = 1.0
        kd = np.zeros(nrw * 128, np.int32)
        pos = 0
        for k, r in rag_cls:
            kd[pos : pos + r] = k
            pos += r
        real = kd > 0
        rt[real, 0] = 1.0 / kd[real]
        rt[real, 1] = (rag_kw[real] - kd[real]).astype(np.float32)

    # ---- device ---------------------------------------------------------
    nc = _build_nc(full_cls, rag_win, ndcp, s_elems, nwt, nrw)
    in_maps = []
    for c in range(NCORES):
        m = {"slots": bufs[c], "wts": wts, "b": b_col}
        if nrw:
            m["rt"] = rt
        in_maps.append(m)
    trace = os.environ.get("KERNEL_TRACE", "0") == "1"
    kw_ = {}
    if trace:
        import tempfile
        kw_ = dict(trace=True, tmpdir=tempfile.mkdtemp(prefix="kernel_trace_"))
    res = run_bass_kernel_spmd(nc, in_maps, core_ids=list(range(NCORES)), **kw_)
    last_exec_ns = res.exec_time_ns

    # ---- host-side unpermute -------------------------------------------
    outs = np.stack([res.results[c]["out"] for c in range(NCORES)])  # (8, D, ndcp)
    full = np.empty((nd, D), np.float32)
    real = dom_core >= 0
    full[real] = outs[dom_core[real], :, dom_col[real]]
    if n0:
        full[~real] = b
    return full



# revision 5
# speedup vs baseline: 2.1103x; 2.1103x over previous
"""DOM pooling (segment mean+max over pulses, then linear projection) on 8 trn2 cores.

Strategy (v2, fp16 feature-major):
  Host: bucket DOMs by exact pulse count k ("classes"); deal DOMs of each
  class round-robin across the 8 cores so every core has identical structure
  (per-class m = ceil(n_k/8) doms, zero-padded). Windows of 128 doms per
  class, organized as two halves of <=64 doms. Slot buffers are fp16,
  feature-major: partition p = half*64 + embed, free = (window, slot, dom).
  Partial windows use D_w = ceil(rem/2) dom columns per half. No ragged
  sharing, no pad corrections (padding is all-zeros -> dummy doms only).

  Device (one NEFF, SPMD on 8 cores), per class window-group:
    - one contiguous fp16 DMA load per group (up to 32KB/partition runs)
    - segment SUM fused into the projection: k accumulating PE matmuls with
      block-diagonal fp16 weights blkdiag(W_sum/k) -> PSUM holds the
      projected mean contribution (scaling folded into weights)
    - segment MAX as a pairwise fp16 tensor_tensor tree on DVE (2x mode),
      then one more accumulating matmul with blkdiag(W_max) closes PSUM
    - ACT adds bias during PSUM->SBUF copy (fp16 out); per-group DMA store
  Partial windows compute both sum and max trees on DVE (tiny) + 2 matmuls.

  Host: scatter per-core outputs [128=(half,e), cols] back to (num_doms, 64).
"""
import sys

import numpy as np

for _p in ("/opt/trn_rl_repo",):
    if _p not in sys.path:
        sys.path.append(_p)

from concourse import bacc
import concourse.mybir as mybir
import concourse.tile as tile
from concourse.bass_utils import run_bass_kernel_spmd

NCORES = 8
D = 64
FP32 = mybir.dt.float32
FP16 = mybir.dt.float16

last_exec_ns = None  # set when KERNEL_TRACE=1


def _plan(counts):
    """Shared class/window structure (derived from global counts).

    Returns list of class dicts and totals. Per class k:
      m: doms per core (ceil), fw: full 128-dom windows, rem: leftover doms,
      dw: per-half columns of the partial window (ceil(rem/2)),
      base_f/base_p: slot-buffer element offsets (full / partial region),
      hcol0: first output column (per-half column space).
    """
    kmax = int(counts.max()) if counts.size else 0
    n_k = np.bincount(counts, minlength=kmax + 1)
    classes = []
    base = 0
    hcol = 0
    for k in range(1, kmax + 1):
        if n_k[k] == 0:
            continue
        m = -(-int(n_k[k]) // NCORES)
        fw = m // 128
        rem = m % 128
        dw = -(-rem // 2)
        c = dict(k=k, n=int(n_k[k]), m=m, fw=fw, rem=rem, dw=dw,
                 base_f=base, hcol0=hcol)
        base += 128 * fw * k * D
        hcol += fw * D
        c["base_p"] = base
        if rem:
            base += 128 * k * dw
            hcol += dw
        classes.append(c)
    return classes, base, hcol


def _emit_tree(eng, nc, v, k, gwa, dcols, out4, op, workp, tagp):
    """Pairwise reduction tree over the slot axis.

    v: 4D view (p, w=gwa, s=k, d=dcols); out4: (p, w, 1, d) destination view.
    Emits ceil(log2 k) fp16 tensor_tensor levels (+copies for odd carries).
    Caller must handle k == 1 (no op needed).
    """
    assert k >= 2
    cur = v
    s = k
    lvl = 0
    while s > 1:
        b = s // 2
        odd = s & 1
        tgt = b + odd
        if b == 1 and odd == 0:
            eng.tensor_tensor(out=out4[:, :, 0:1, :], in0=cur[:, :, 0:1, :],
                              in1=cur[:, :, 1:2, :], op=op)
            return
        wt = workp.tile([128, gwa * tgt * dcols], FP16, tag=f"{tagp}{lvl % 2}")
        dst = wt[:].rearrange("p (w s d) -> p w s d", w=gwa, s=tgt)
        eng.tensor_tensor(out=dst[:, :, 0:b, :], in0=cur[:, :, 0:b, :],
                          in1=cur[:, :, b : 2 * b, :], op=op)
        if odd:
            eng.tensor_copy(dst[:, :, b : b + 1, :], cur[:, :, 2 * b : 2 * b + 1, :])
        cur = dst
        s = tgt
        lvl += 1


def _build_nc(classes, s_elems, ncolh):
    ncls = len(classes)
    nc = bacc.Bacc(None)
    slots_t = nc.dram_tensor("slots", [s_elems], FP16, kind="ExternalInput")
    wts_t = nc.dram_tensor("wts", [(ncls + 1) * 128, 128], FP16, kind="ExternalInput")
    b_t = nc.dram_tensor("b", [128, 1], FP32, kind="ExternalInput")
    out_t = nc.dram_tensor("out", [128, ncolh], FP16, kind="ExternalOutput")

    with tile.TileContext(nc) as tc:
        with (
            tc.tile_pool(name="const", bufs=1) as constp,
            tc.tile_pool(name="inp", bufs=3) as inp,
            tc.tile_pool(name="work", bufs=2) as workp,
            tc.tile_pool(name="redg", bufs=3) as redp,
            tc.tile_pool(name="outp", bufs=3) as outp,
            tc.tile_pool(name="ps", bufs=4, space="PSUM") as psp,
        ):
            wk_sb = constp.tile([128, (ncls + 1) * 128], FP16)
            nc.sync.dma_start(
                wk_sb[:].rearrange("p (j m) -> p j m", m=128),
                wts_t[:, :].rearrange("(j p) m -> p j m", p=128),
            )
            b_sb = constp.tile([128, 1], FP32)
            nc.sync.dma_start(b_sb[:], b_t[:])

            def w_ap(j):
                return wk_sb[:, j * 128 : (j + 1) * 128]

            wmax_j = ncls  # last weight block = blkdiag(W_max), unscaled

            def do_windows(jcls, k, in_t, gwa, dcols, base_col):
                """Reduce+project gwa windows of dcols half-columns each."""
                v = in_t[:].rearrange("p (w s d) -> p w s d", w=gwa, s=k)
                use_tree = k > 1
                if use_tree:
                    maxg = redp.tile([128, gwa * dcols], FP16, tag="maxg")
                    mg4 = maxg[:].rearrange("p (w s d) -> p w s d", w=gwa, s=1)
                    mg3 = mg4[:, :, 0, :]
                    _emit_tree(nc.vector, nc, v, k, gwa, dcols, mg4,
                               mybir.AluOpType.max, workp, "m")
                out_sb = outp.tile([128, gwa * dcols], FP16, tag="out")
                for w0 in range(0, gwa, 8):
                    sgw = min(8, gwa - w0)
                    N = sgw * dcols
                    ps = psp.tile([128, N], FP32, space="PSUM", tag="ps")
                    for s in range(k):
                        nc.tensor.matmul(
                            ps[:, :N], lhsT=w_ap(jcls),
                            rhs=v[:, w0 : w0 + sgw, s, :],
                            start=(s == 0), stop=False,
                        )
                    if use_tree:
                        rhs_max = mg3[:, w0 : w0 + sgw, :]
                    else:
                        rhs_max = v[:, w0 : w0 + sgw, 0, :]
                    nc.tensor.matmul(
                        ps[:, :N], lhsT=w_ap(wmax_j), rhs=rhs_max,
                        start=False, stop=True,
                    )
                    nc.scalar.activation(
                        out_sb[:, w0 * dcols : w0 * dcols + N], ps[:, :N],
                        mybir.ActivationFunctionType.Identity, bias=b_sb[:, :1],
                    )
                nc.sync.dma_start(
                    out_t[:, base_col : base_col + gwa * dcols],
                    out_sb[:, : gwa * dcols],
                )

            for jcls, c in enumerate(classes):
                k, fw, rem, dw = c["k"], c["fw"], c["rem"], c["dw"]
                gw = max(1, min(256 // k, 64))
                F_cls = fw * k * D
                if fw:
                    full2d = slots_t[c["base_f"] : c["base_f"] + 128 * F_cls] \
                        .rearrange("(p f) -> p f", p=128)
                    for g0 in range(0, fw, gw):
                        gwa = min(gw, fw - g0)
                        F0 = gwa * k * D
                        in_t = inp.tile([128, F0], FP16, tag="in")
                        nc.sync.dma_start(
                            in_t[:], full2d[:, g0 * k * D : g0 * k * D + F0]
                        )
                        do_windows(jcls, k, in_t, gwa, D,
                                   c["hcol0"] + g0 * D)
                if rem:
                    F0 = k * dw
                    in_t = inp.tile([128, F0], FP16, tag="in")
                    nc.sync.dma_start(
                        in_t[:],
                        slots_t[c["base_p"] : c["base_p"] + 128 * F0]
                        .rearrange("(p f) -> p f", p=128),
                    )
                    do_windows(jcls, k, in_t, 1, dw, c["hcol0"] + fw * D)
    nc.finalize()
    return nc


def kernel(pulse_embeddings, pulse_to_dom_idx, num_doms, proj_w, proj_b):
    global last_exec_ns
    import os

    E = np.asarray(pulse_embeddings, dtype=np.float32)
    E16 = E.astype(np.float16)
    idx = np.asarray(pulse_to_dom_idx).astype(np.int64)
    nd = int(num_doms)
    W = np.asarray(proj_w, dtype=np.float32)   # (D, 2D)
    b = np.asarray(proj_b, dtype=np.float32)   # (D,)

    counts = np.bincount(idx, minlength=nd)
    classes, s_elems, ncolh = _plan(counts)
    ncls = len(classes)

    dom_order = np.argsort(counts, kind="stable")
    n0 = int((counts == 0).sum())
    perm = np.argsort(idx, kind="stable")
    pstart = np.zeros(nd + 1, np.int64)
    pstart[1:] = np.cumsum(counts)

    # per-dom output routing (core, half, halfcol) for real doms
    dom_core = np.full(nd, -1, np.int32)
    dom_half = np.zeros(nd, np.int32)
    dom_hcol = np.zeros(nd, np.int32)

    bufs = [np.zeros(s_elems, np.float16) for _ in range(NCORES)]
    off = n0
    for c in classes:
        k, n, m, fw, rem, dw = c["k"], c["n"], c["m"], c["fw"], c["rem"], c["dw"]
        doms_k = dom_order[off : off + n]
        off += n
        # routing: class-list index i -> core i%8, position p=i//8
        i_arr = np.arange(n, dtype=np.int64)
        p_arr = i_arr // NCORES
        dom_core[doms_k] = (i_arr % NCORES).astype(np.int32)
        isfull = p_arr < fw * 128
        q = np.where(isfull, p_arr % 128, p_arr - fw * 128)
        halfsz = np.where(isfull, 64, dw)
        dom_half[doms_k] = (q // halfsz).astype(np.int32)
        dcol = q % halfsz
        dom_hcol[doms_k] = np.where(
            isfull, c["hcol0"] + (p_arr // 128) * D + dcol,
            c["hcol0"] + fw * D + dcol,
        ).astype(np.int32)

        for cc in range(NCORES):
            doms_c = doms_k[cc::NCORES]
            nreal = len(doms_c)
            rows = pstart[doms_c][:, None] + np.arange(k)[None, :]
            X = E16[perm[rows]]  # (nreal, k, 64)
            if nreal < m:
                X = np.concatenate(
                    [X, np.zeros((m - nreal, k, D), np.float16)], axis=0
                )
            if fw:
                Xf = X[: fw * 128].reshape(fw, 2, 64, k, D)  # w h d s e
                arr = Xf.transpose(1, 4, 0, 3, 2)            # h e w s d
                bufs[cc][c["base_f"] : c["base_f"] + 128 * fw * k * D] = \
                    np.ascontiguousarray(arr).reshape(-1)
            if rem:
                Xr = X[fw * 128 :]  # (rem, k, D)
                if rem < 2 * dw:
                    Xr = np.concatenate(
                        [Xr, np.zeros((2 * dw - rem, k, D), np.float16)], axis=0
                    )
                arr = Xr.reshape(2, dw, k, D).transpose(0, 3, 2, 1)  # h e s d
                bufs[cc][c["base_p"] : c["base_p"] + 128 * k * dw] = \
                    np.ascontiguousarray(arr).reshape(-1)

    # ---- weights: per-class blkdiag(W_sum/k), plus blkdiag(W_max) --------
    Wsum = W[:, :D]   # (out_e, feat_e)
    Wmax = W[:, D:]
    wts = np.zeros(((ncls + 1) * 128, 128), np.float16)
    for j, c in enumerate(classes):
        blk = (Wsum.T / np.float32(c["k"])).astype(np.float16)  # (feat, out)
        wts[j * 128 : j * 128 + 64, 0:64] = blk
        wts[j * 128 + 64 : (j + 1) * 128, 64:128] = blk
    blk = Wmax.T.astype(np.float16)
    wts[ncls * 128 : ncls * 128 + 64, 0:64] = blk
    wts[ncls * 128 + 64 :, 64:128] = blk
    b_col = np.tile(b, 2).reshape(128, 1).astype(np.float32)

    # ---- device ----------------------------------------------------------
    nc = _build_nc(classes, s_elems, ncolh)
    in_maps = [{"slots": bufs[cc], "wts": wts, "b": b_col} for cc in range(NCORES)]
    trace = os.environ.get("KERNEL_TRACE", "0") == "1"
    kw_ = {}
    if trace:
        import tempfile
        kw_ = dict(trace=True, tmpdir=tempfile.mkdtemp(prefix="kernel_trace_"))
    res = run_bass_kernel_spmd(nc, in_maps, core_ids=list(range(NCORES)), **kw_)
    last_exec_ns = res.exec_time_ns

    # ---- host-side unpermute --------------------------------------------
    outs = np.stack([res.results[cc]["out"] for cc in range(NCORES)]) \
        .astype(np.float32)  # (8, 128, ncolh)
    full = np.empty((nd, D), np.float32)
    real = dom_core >= 0
    rc = dom_core[real]
    rh = dom_half[real]
    rcol = dom_hcol[real]
    rows = rh[:, None] * D + np.arange(D)[None, :]
    full[real] = outs[rc[:, None], rows, rcol[:, None]]
    if n0:
        full[~real] = b
    return full


# revision 7
# speedup vs baseline: 2.4059x; 1.1401x over previous
"""DOM pooling (segment mean+max over pulses, then linear projection) on 8 trn2 cores.

Strategy (v2, fp16 feature-major):
  Host: bucket DOMs by exact pulse count k ("classes"); deal DOMs of each
  class round-robin across the 8 cores so every core has identical structure
  (per-class m = ceil(n_k/8) doms, zero-padded). Windows of 128 doms per
  class, organized as two halves of <=64 doms. Slot buffers are fp16,
  feature-major: partition p = half*64 + embed, free = (window, slot, dom).
  Partial windows use D_w = ceil(rem/2) dom columns per half. No ragged
  sharing, no pad corrections (padding is all-zeros -> dummy doms only).

  Device (one NEFF, SPMD on 8 cores), per class window-group:
    - one contiguous fp16 DMA load per group (up to 32KB/partition runs)
    - segment SUM fused into the projection: k accumulating PE matmuls with
      block-diagonal fp16 weights blkdiag(W_sum/k) -> PSUM holds the
      projected mean contribution (scaling folded into weights)
    - segment MAX as a pairwise fp16 tensor_tensor tree on DVE (2x mode),
      then one more accumulating matmul with blkdiag(W_max) closes PSUM
    - ACT adds bias during PSUM->SBUF copy (fp16 out); per-group DMA store
  Partial windows compute both sum and max trees on DVE (tiny) + 2 matmuls.

  Host: scatter per-core outputs [128=(half,e), cols] back to (num_doms, 64).
"""
import sys

import numpy as np

for _p in ("/opt/trn_rl_repo",):
    if _p not in sys.path:
        sys.path.append(_p)

from concourse import bacc
import concourse.mybir as mybir
import concourse.tile as tile
from concourse.bass_utils import run_bass_kernel_spmd

NCORES = 8
D = 64
FP32 = mybir.dt.float32
FP16 = mybir.dt.float16

last_exec_ns = None  # set when KERNEL_TRACE=1


def _plan(counts):
    """Shared class/window structure (derived from global counts).

    Returns list of class dicts and totals. Per class k:
      m: doms per core (ceil), fw: full 128-dom windows, rem: leftover doms,
      dw: per-half columns of the partial window (ceil(rem/2)),
      base_f/base_p: slot-buffer element offsets (full / partial region),
      hcol0: first output column (per-half column space).
    """
    kmax = int(counts.max()) if counts.size else 0
    n_k = np.bincount(counts, minlength=kmax + 1)
    classes = []
    base = 0
    hcol = 0
    for k in range(1, kmax + 1):
        if n_k[k] == 0:
            continue
        m = -(-int(n_k[k]) // NCORES)
        fw = m // 128
        rem = m % 128
        dw = -(-rem // 2)
        c = dict(k=k, n=int(n_k[k]), m=m, fw=fw, rem=rem, dw=dw,
                 base_f=base, hcol0=hcol)
        base += 128 * fw * k * D
        hcol += fw * D
        c["base_p"] = base
        if rem:
            base += 128 * k * dw
            hcol += dw
        classes.append(c)
    return classes, base, hcol


def _emit_tree(eng, nc, v, k, gwa, dcols, out4, op, workp, tagp):
    """Pairwise reduction tree over the slot axis.

    v: 4D view (p, w=gwa, s=k, d=dcols); out4: (p, w, 1, d) destination view.
    Emits ceil(log2 k) fp16 tensor_tensor levels (+copies for odd carries).
    Caller must handle k == 1 (no op needed).
    """
    assert k >= 2
    cur = v
    s = k
    lvl = 0
    while s > 1:
        b = s // 2
        odd = s & 1
        tgt = b + odd
        if b == 1 and odd == 0:
            eng.tensor_tensor(out=out4[:, :, 0:1, :], in0=cur[:, :, 0:1, :],
                              in1=cur[:, :, 1:2, :], op=op)
            return
        wt = workp.tile([128, gwa * tgt * dcols], FP16, tag=f"{tagp}{lvl % 2}")
        dst = wt[:].rearrange("p (w s d) -> p w s d", w=gwa, s=tgt)
        eng.tensor_tensor(out=dst[:, :, 0:b, :], in0=cur[:, :, 0:b, :],
                          in1=cur[:, :, b : 2 * b, :], op=op)
        if odd:
            eng.tensor_copy(dst[:, :, b : b + 1, :], cur[:, :, 2 * b : 2 * b + 1, :])
        cur = dst
        s = tgt
        lvl += 1


def _build_nc(classes, s_elems, ncolh):
    ncls = len(classes)
    nc = bacc.Bacc(None)
    slots_t = nc.dram_tensor("slots", [s_elems], FP16, kind="ExternalInput")
    wts_t = nc.dram_tensor("wts", [(ncls + 1) * 128, 128], FP16, kind="ExternalInput")
    b_t = nc.dram_tensor("b", [128, 1], FP32, kind="ExternalInput")
    out_t = nc.dram_tensor("out", [128, ncolh], FP16, kind="ExternalOutput")

    with tile.TileContext(nc) as tc:
        with (
            tc.tile_pool(name="const", bufs=1) as constp,
            tc.tile_pool(name="inp", bufs=4) as inp,
            tc.tile_pool(name="work", bufs=1) as workp,
            tc.tile_pool(name="redg", bufs=2) as redp,
            tc.tile_pool(name="outp", bufs=2) as outp,
            tc.tile_pool(name="ps", bufs=4, space="PSUM") as psp,
        ):
            wk_sb = constp.tile([128, (ncls + 1) * 128], FP16)
            nc.sync.dma_start(
                wk_sb[:].rearrange("p (j m) -> p j m", m=128),
                wts_t[:, :].rearrange("(j p) m -> p j m", p=128),
            )
            b_sb = constp.tile([128, 1], FP32)
            nc.sync.dma_start(b_sb[:], b_t[:])

            def w_ap(j):
                return wk_sb[:, j * 128 : (j + 1) * 128]

            wmax_j = ncls  # last weight block = blkdiag(W_max), unscaled

            def do_windows(jcls, k, in_t, gwa, dcols, base_col):
                """Reduce+project gwa windows of dcols half-columns each."""
                v = in_t[:].rearrange("p (w s d) -> p w s d", w=gwa, s=k)
                use_tree = k > 1
                if use_tree:
                    maxg = redp.tile([128, gwa * dcols], FP16, tag="maxg")
                    mg4 = maxg[:].rearrange("p (w s d) -> p w s d", w=gwa, s=1)
                    mg3 = mg4[:, :, 0, :]
                    _emit_tree(nc.vector, nc, v, k, gwa, dcols, mg4,
                               mybir.AluOpType.max, workp, "m")
                out_sb = outp.tile([128, gwa * dcols], FP16, tag="out")
                for w0 in range(0, gwa, 8):
                    sgw = min(8, gwa - w0)
                    N = sgw * dcols
                    ps = psp.tile([128, N], FP32, space="PSUM", tag="ps")
                    for s in range(k):
                        nc.tensor.matmul(
                            ps[:, :N], lhsT=w_ap(jcls),
                            rhs=v[:, w0 : w0 + sgw, s, :],
                            start=(s == 0), stop=False,
                        )
                    if use_tree:
                        rhs_max = mg3[:, w0 : w0 + sgw, :]
                    else:
                        rhs_max = v[:, w0 : w0 + sgw, 0, :]
                    nc.tensor.matmul(
                        ps[:, :N], lhsT=w_ap(wmax_j), rhs=rhs_max,
                        start=False, stop=True,
                    )
                    nc.scalar.activation(
                        out_sb[:, w0 * dcols : w0 * dcols + N], ps[:, :N],
                        mybir.ActivationFunctionType.Identity, bias=b_sb[:, :1],
                    )
                nc.gpsimd.dma_start(
                    out_t[:, base_col : base_col + gwa * dcols],
                    out_sb[:, : gwa * dcols],
                )

            # full-window groups first (largest classes first), then the
            # small partial windows to fill the pipeline tail
            order = sorted(range(len(classes)),
                           key=lambda jj: -classes[jj]["fw"] * classes[jj]["k"])
            for jcls in order:
                c = classes[jcls]
                k, fw = c["k"], c["fw"]
                if not fw:
                    continue
                gw = max(1, 192 // k)
                F_cls = fw * k * D
                full2d = slots_t[c["base_f"] : c["base_f"] + 128 * F_cls] \
                    .rearrange("(p f) -> p f", p=128)
                for g0 in range(0, fw, gw):
                    gwa = min(gw, fw - g0)
                    F0 = gwa * k * D
                    in_t = inp.tile([128, F0], FP16, tag="in")
                    nc.sync.dma_start(
                        in_t[:], full2d[:, g0 * k * D : g0 * k * D + F0]
                    )
                    do_windows(jcls, k, in_t, gwa, D, c["hcol0"] + g0 * D)
            for jcls, c in enumerate(classes):
                k, fw, rem, dw = c["k"], c["fw"], c["rem"], c["dw"]
                if not rem:
                    continue
                F0 = k * dw
                in_t = inp.tile([128, F0], FP16, tag="in")
                nc.sync.dma_start(
                    in_t[:],
                    slots_t[c["base_p"] : c["base_p"] + 128 * F0]
                    .rearrange("(p f) -> p f", p=128),
                )
                do_windows(jcls, k, in_t, 1, dw, c["hcol0"] + fw * D)
    nc.finalize()
    return nc


def kernel(pulse_embeddings, pulse_to_dom_idx, num_doms, proj_w, proj_b):
    global last_exec_ns
    import os

    E = np.asarray(pulse_embeddings, dtype=np.float32)
    E16 = E.astype(np.float16)
    idx = np.asarray(pulse_to_dom_idx).astype(np.int64)
    nd = int(num_doms)
    W = np.asarray(proj_w, dtype=np.float32)   # (D, 2D)
    b = np.asarray(proj_b, dtype=np.float32)   # (D,)

    counts = np.bincount(idx, minlength=nd)
    classes, s_elems, ncolh = _plan(counts)
    ncls = len(classes)

    dom_order = np.argsort(counts, kind="stable")
    n0 = int((counts == 0).sum())
    perm = np.argsort(idx, kind="stable")
    pstart = np.zeros(nd + 1, np.int64)
    pstart[1:] = np.cumsum(counts)

    # per-dom output routing (core, half, halfcol) for real doms
    dom_core = np.full(nd, -1, np.int32)
    dom_half = np.zeros(nd, np.int32)
    dom_hcol = np.zeros(nd, np.int32)

    bufs = [np.zeros(s_elems, np.float16) for _ in range(NCORES)]
    off = n0
    for c in classes:
        k, n, m, fw, rem, dw = c["k"], c["n"], c["m"], c["fw"], c["rem"], c["dw"]
        doms_k = dom_order[off : off + n]
        off += n
        # routing: class-list index i -> core i%8, position p=i//8
        i_arr = np.arange(n, dtype=np.int64)
        p_arr = i_arr // NCORES
        dom_core[doms_k] = (i_arr % NCORES).astype(np.int32)
        isfull = p_arr < fw * 128
        q = np.where(isfull, p_arr % 128, p_arr - fw * 128)
        halfsz = np.where(isfull, 64, dw)
        dom_half[doms_k] = (q // halfsz).astype(np.int32)
        dcol = q % halfsz
        dom_hcol[doms_k] = np.where(
            isfull, c["hcol0"] + (p_arr // 128) * D + dcol,
            c["hcol0"] + fw * D + dcol,
        ).astype(np.int32)

        for cc in range(NCORES):
            doms_c = doms_k[cc::NCORES]
            nreal = len(doms_c)
            rows = pstart[doms_c][:, None] + np.arange(k)[None, :]
            X = E16[perm[rows]]  # (nreal, k, 64)
            if nreal < m:
                X = np.concatenate(
                    [X, np.zeros((m - nreal, k, D), np.float16)], axis=0
                )
            if fw:
                Xf = X[: fw * 128].reshape(fw, 2, 64, k, D)  # w h d s e
                arr = Xf.transpose(1, 4, 0, 3, 2)            # h e w s d
                bufs[cc][c["base_f"] : c["base_f"] + 128 * fw * k * D] = \
                    np.ascontiguousarray(arr).reshape(-1)
            if rem:
                Xr = X[fw * 128 :]  # (rem, k, D)
                if rem < 2 * dw:
                    Xr = np.concatenate(
                        [Xr, np.zeros((2 * dw - rem, k, D), np.float16)], axis=0
                    )
                arr = Xr.reshape(2, dw, k, D).transpose(0, 3, 2, 1)  # h e s d
                bufs[cc][c["base_p"] : c["base_p"] + 128 * k * dw] = \
                    np.ascontiguousarray(arr).reshape(-1)

    # ---- weights: per-class blkdiag(W_sum/k), plus blkdiag(W_max) --------
    Wsum = W[:, :D]   # (out_e, feat_e)
    Wmax = W[:, D:]
    wts = np.zeros(((ncls + 1) * 128, 128), np.float16)
    for j, c in enumerate(classes):
        blk = (Wsum.T / np.float32(c["k"])).astype(np.float16)  # (feat, out)
        wts[j * 128 : j * 128 + 64, 0:64] = blk
        wts[j * 128 + 64 : (j + 1) * 128, 64:128] = blk
    blk = Wmax.T.astype(np.float16)
    wts[ncls * 128 : ncls * 128 + 64, 0:64] = blk
    wts[ncls * 128 + 64 :, 64:128] = blk
    b_col = np.tile(b, 2).reshape(128, 1).astype(np.float32)

    # ---- device ----------------------------------------------------------
    nc = _build_nc(classes, s_elems, ncolh)
    in_maps = [{"slots": bufs[cc], "wts": wts, "b": b_col} for cc in range(NCORES)]
    trace = os.environ.get("KERNEL_TRACE", "0") == "1"
    kw_ = {}
    if trace:
        import tempfile
        kw_ = dict(trace=True, tmpdir=tempfile.mkdtemp(prefix="kernel_trace_"))
    res = run_bass_kernel_spmd(nc, in_maps, core_ids=list(range(NCORES)), **kw_)
    last_exec_ns = res.exec_time_ns

    # ---- host-side unpermute --------------------------------------------
    outs = np.stack([res.results[cc]["out"] for cc in range(NCORES)]) \
        .astype(np.float32)  # (8, 128, ncolh)
    full = np.empty((nd, D), np.float32)
    real = dom_core >= 0
    rc = dom_core[real]
    rh = dom_half[real]
    rcol = dom_hcol[real]
    rows = rh[:, None] * D + np.arange(D)[None, :]
    full[real] = outs[rc[:, None], rows, rcol[:, None]]
    if n0:
        full[~real] = b
    return full


# revision 9
# speedup vs baseline: 2.6545x; 1.1033x over previous
"""DOM pooling (segment mean+max over pulses, then linear projection) on 8 trn2 cores.

Strategy (v2, fp16 feature-major):
  Host: bucket DOMs by exact pulse count k ("classes"); deal DOMs of each
  class round-robin across the 8 cores so every core has identical structure
  (per-class m = ceil(n_k/8) doms, zero-padded). Windows of 128 doms per
  class, organized as two halves of <=64 doms. Slot buffers are fp16,
  feature-major: partition p = half*64 + embed, free = (window, slot, dom).
  Partial windows use D_w = ceil(rem/2) dom columns per half. No ragged
  sharing, no pad corrections (padding is all-zeros -> dummy doms only).

  Device (one NEFF, SPMD on 8 cores), per class window-group:
    - one contiguous fp16 DMA load per group (up to 32KB/partition runs)
    - segment SUM fused into the projection: k accumulating PE matmuls with
      block-diagonal fp16 weights blkdiag(W_sum/k) -> PSUM holds the
      projected mean contribution (scaling folded into weights)
    - segment MAX as a pairwise fp16 tensor_tensor tree on DVE (2x mode),
      then one more accumulating matmul with blkdiag(W_max) closes PSUM
    - ACT adds bias during PSUM->SBUF copy (fp16 out); per-group DMA store
  Partial windows compute both sum and max trees on DVE (tiny) + 2 matmuls.

  Host: scatter per-core outputs [128=(half,e), cols] back to (num_doms, 64).
"""
import sys

import numpy as np

for _p in ("/opt/trn_rl_repo",):
    if _p not in sys.path:
        sys.path.append(_p)

from concourse import bacc
import concourse.mybir as mybir
import concourse.tile as tile
from concourse.bass_utils import run_bass_kernel_spmd

NCORES = 8
D = 64
FP32 = mybir.dt.float32
FP16 = mybir.dt.float16

last_exec_ns = None  # set when KERNEL_TRACE=1


def _plan(counts):
    """Shared class/window structure (derived from global counts).

    Returns list of class dicts and totals. Per class k:
      m: doms per core (ceil), fw: full 128-dom windows, rem: leftover doms,
      dw: per-half columns of the partial window (ceil(rem/2)),
      base_f/base_p: slot-buffer element offsets (full / partial region),
      hcol0: first output column (per-half column space).
    """
    kmax = int(counts.max()) if counts.size else 0
    n_k = np.bincount(counts, minlength=kmax + 1)
    classes = []
    base = 0
    hcol = 0
    for k in range(1, kmax + 1):
        if n_k[k] == 0:
            continue
        m = -(-int(n_k[k]) // NCORES)
        fw = m // 128
        rem = m % 128
        dw = -(-rem // 2)
        c = dict(k=k, n=int(n_k[k]), m=m, fw=fw, rem=rem, dw=dw,
                 base_f=base, hcol0=hcol)
        base += 128 * fw * k * D
        hcol += fw * D
        c["base_p"] = base
        if rem:
            base += 128 * k * dw
            hcol += dw
        classes.append(c)
    return classes, base, hcol


def _emit_tree(eng, nc, v, k, gwa, dcols, out4, op, workp, tagp):
    """Pairwise reduction tree over the slot axis.

    v: 4D view (p, w=gwa, s=k, d=dcols); out4: (p, w, 1, d) destination view.
    Emits ceil(log2 k) fp16 tensor_tensor levels (+copies for odd carries).
    Caller must handle k == 1 (no op needed).
    """
    assert k >= 2
    cur = v
    s = k
    lvl = 0
    while s > 1:
        b = s // 2
        odd = s & 1
        tgt = b + odd
        if b == 1 and odd == 0:
            eng.tensor_tensor(out=out4[:, :, 0:1, :], in0=cur[:, :, 0:1, :],
                              in1=cur[:, :, 1:2, :], op=op)
            return
        wt = workp.tile([128, gwa * tgt * dcols], FP16, tag=f"{tagp}{lvl % 2}")
        dst = wt[:].rearrange("p (w s d) -> p w s d", w=gwa, s=tgt)
        eng.tensor_tensor(out=dst[:, :, 0:b, :], in0=cur[:, :, 0:b, :],
                          in1=cur[:, :, b : 2 * b, :], op=op)
        if odd:
            eng.tensor_copy(dst[:, :, b : b + 1, :], cur[:, :, 2 * b : 2 * b + 1, :])
        cur = dst
        s = tgt
        lvl += 1


def _build_nc(classes, s_elems, ncolh):
    ncls = len(classes)
    nc = bacc.Bacc(None)
    slots_t = nc.dram_tensor("slots", [s_elems], FP16, kind="ExternalInput")
    wts_t = nc.dram_tensor("wts", [(ncls + 1) * 128, 128], FP16, kind="ExternalInput")
    b_t = nc.dram_tensor("b", [128, 1], FP32, kind="ExternalInput")
    out_t = nc.dram_tensor("out", [128, ncolh], FP16, kind="ExternalOutput")

    with tile.TileContext(nc) as tc:
        with (
            tc.tile_pool(name="const", bufs=1) as constp,
            tc.tile_pool(name="inp", bufs=4) as inp,
            tc.tile_pool(name="pin", bufs=4) as pinp,
            tc.tile_pool(name="work", bufs=1) as workp,
            tc.tile_pool(name="redg", bufs=2) as redp,
            tc.tile_pool(name="outp", bufs=2) as outp,
            tc.tile_pool(name="ps", bufs=4, space="PSUM") as psp,
        ):
            wk_sb = constp.tile([128, (ncls + 1) * 128], FP16)
            nc.sync.dma_start(
                wk_sb[:].rearrange("p (j m) -> p j m", m=128),
                wts_t[:, :].rearrange("(j p) m -> p j m", p=128),
            )
            b_sb = constp.tile([128, 1], FP32)
            nc.sync.dma_start(b_sb[:], b_t[:])

            def w_ap(j):
                return wk_sb[:, j * 128 : (j + 1) * 128]

            wmax_j = ncls  # last weight block = blkdiag(W_max), unscaled

            def do_windows(jcls, k, in_t, gwa, dcols, base_col):
                """Reduce+project gwa windows of dcols half-columns each."""
                v = in_t[:].rearrange("p (w s d) -> p w s d", w=gwa, s=k)
                use_tree = k > 1
                if use_tree:
                    maxg = redp.tile([128, gwa * dcols], FP16, tag="maxg")
                    mg4 = maxg[:].rearrange("p (w s d) -> p w s d", w=gwa, s=1)
                    mg3 = mg4[:, :, 0, :]
                    _emit_tree(nc.vector, nc, v, k, gwa, dcols, mg4,
                               mybir.AluOpType.max, workp, "m")
                out_sb = outp.tile([128, gwa * dcols], FP16, tag="out")
                for w0 in range(0, gwa, 8):
                    sgw = min(8, gwa - w0)
                    N = sgw * dcols
                    ps = psp.tile([128, N], FP32, space="PSUM", tag="ps")
                    for s in range(k):
                        nc.tensor.matmul(
                            ps[:, :N], lhsT=w_ap(jcls),
                            rhs=v[:, w0 : w0 + sgw, s, :],
                            start=(s == 0), stop=False,
                        )
                    if use_tree:
                        rhs_max = mg3[:, w0 : w0 + sgw, :]
                    else:
                        rhs_max = v[:, w0 : w0 + sgw, 0, :]
                    nc.tensor.matmul(
                        ps[:, :N], lhsT=w_ap(wmax_j), rhs=rhs_max,
                        start=False, stop=True,
                    )
                    nc.scalar.activation(
                        out_sb[:, w0 * dcols : w0 * dcols + N], ps[:, :N],
                        mybir.ActivationFunctionType.Identity, bias=b_sb[:, :1],
                    )
                nc.gpsimd.dma_start(
                    out_t[:, base_col : base_col + gwa * dcols],
                    out_sb[:, : gwa * dcols],
                )

            def emit_partial(jcls):
                c = classes[jcls]
                k, fw, dw = c["k"], c["fw"], c["dw"]
                F0 = k * dw
                in_t = pinp.tile([128, F0], FP16, tag="pin")
                nc.sync.dma_start(
                    in_t[:],
                    slots_t[c["base_p"] : c["base_p"] + 128 * F0]
                    .rearrange("(p f) -> p f", p=128),
                )
                do_windows(jcls, k, in_t, 1, dw, c["hcol0"] + fw * D)

            # full-window groups (largest classes first), with the small
            # partial windows interleaved so their latency-bound chains hide
            # under the DMA-dense phase
            order = sorted(range(len(classes)),
                           key=lambda jj: -classes[jj]["fw"] * classes[jj]["k"])
            partials = [jj for jj, c in enumerate(classes) if c["rem"]]
            pi = 0
            for jcls in order:
                c = classes[jcls]
                k, fw = c["k"], c["fw"]
                if not fw:
                    continue
                gw = max(1, 192 // k)
                F_cls = fw * k * D
                full2d = slots_t[c["base_f"] : c["base_f"] + 128 * F_cls] \
                    .rearrange("(p f) -> p f", p=128)
                for g0 in range(0, fw, gw):
                    gwa = min(gw, fw - g0)
                    F0 = gwa * k * D
                    in_t = inp.tile([128, F0], FP16, tag="in")
                    nc.sync.dma_start(
                        in_t[:], full2d[:, g0 * k * D : g0 * k * D + F0]
                    )
                    do_windows(jcls, k, in_t, gwa, D, c["hcol0"] + g0 * D)
                    if pi < len(partials):
                        emit_partial(partials[pi])
                        pi += 1
            while pi < len(partials):
                emit_partial(partials[pi])
                pi += 1
    nc.finalize()
    return nc


def kernel(pulse_embeddings, pulse_to_dom_idx, num_doms, proj_w, proj_b):
    global last_exec_ns
    import os

    E = np.asarray(pulse_embeddings, dtype=np.float32)
    E16 = E.astype(np.float16)
    idx = np.asarray(pulse_to_dom_idx).astype(np.int64)
    nd = int(num_doms)
    W = np.asarray(proj_w, dtype=np.float32)   # (D, 2D)
    b = np.asarray(proj_b, dtype=np.float32)   # (D,)

    counts = np.bincount(idx, minlength=nd)
    classes, s_elems, ncolh = _plan(counts)
    ncls = len(classes)

    dom_order = np.argsort(counts, kind="stable")
    n0 = int((counts == 0).sum())
    perm = np.argsort(idx, kind="stable")
    pstart = np.zeros(nd + 1, np.int64)
    pstart[1:] = np.cumsum(counts)

    # per-dom output routing (core, half, halfcol) for real doms
    dom_core = np.full(nd, -1, np.int32)
    dom_half = np.zeros(nd, np.int32)
    dom_hcol = np.zeros(nd, np.int32)

    bufs = [np.zeros(s_elems, np.float16) for _ in range(NCORES)]
    off = n0
    for c in classes:
        k, n, m, fw, rem, dw = c["k"], c["n"], c["m"], c["fw"], c["rem"], c["dw"]
        doms_k = dom_order[off : off + n]
        off += n
        # routing: class-list index i -> core i%8, position p=i//8
        i_arr = np.arange(n, dtype=np.int64)
        p_arr = i_arr // NCORES
        dom_core[doms_k] = (i_arr % NCORES).astype(np.int32)
        isfull = p_arr < fw * 128
        q = np.where(isfull, p_arr % 128, p_arr - fw * 128)
        halfsz = np.where(isfull, 64, dw)
        dom_half[doms_k] = (q // halfsz).astype(np.int32)
        dcol = q % halfsz
        dom_hcol[doms_k] = np.where(
            isfull, c["hcol0"] + (p_arr // 128) * D + dcol,
            c["hcol0"] + fw * D + dcol,
        ).astype(np.int32)

        for cc in range(NCORES):
            doms_c = doms_k[cc::NCORES]
            nreal = len(doms_c)
            rows = pstart[doms_c][:, None] + np.arange(k)[None, :]
            X = E16[perm[rows]]  # (nreal, k, 64)
            if nreal < m:
                X = np.concatenate(
                    [X, np.zeros((m - nreal, k, D), np.float16)], axis=0
                )
            if fw:
                Xf = X[: fw * 128].reshape(fw, 2, 64, k, D)  # w h d s e
                arr = Xf.transpose(1, 4, 0, 3, 2)            # h e w s d
                bufs[cc][c["base_f"] : c["base_f"] + 128 * fw * k * D] = \
                    np.ascontiguousarray(arr).reshape(-1)
            if rem:
                Xr = X[fw * 128 :]  # (rem, k, D)
                if rem < 2 * dw:
                    Xr = np.concatenate(
                        [Xr, np.zeros((2 * dw - rem, k, D), np.float16)], axis=0
                    )
                arr = Xr.reshape(2, dw, k, D).transpose(0, 3, 2, 1)  # h e s d
                bufs[cc][c["base_p"] : c["base_p"] + 128 * k * dw] = \
                    np.ascontiguousarray(arr).reshape(-1)

    # ---- weights: per-class blkdiag(W_sum/k), plus blkdiag(W_max) --------
    Wsum = W[:, :D]   # (out_e, feat_e)
    Wmax = W[:, D:]
    wts = np.zeros(((ncls + 1) * 128, 128), np.float16)
    for j, c in enumerate(classes):
        blk = (Wsum.T / np.float32(c["k"])).astype(np.float16)  # (feat, out)
        wts[j * 128 : j * 128 + 64, 0:64] = blk
        wts[j * 128 + 64 : (j + 1) * 128, 64:128] = blk
    blk = Wmax.T.astype(np.float16)
    wts[ncls * 128 : ncls * 128 + 64, 0:64] = blk
    wts[ncls * 128 + 64 :, 64:128] = blk
    b_col = np.tile(b, 2).reshape(128, 1).astype(np.float32)

    # ---- device ----------------------------------------------------------
    nc = _build_nc(classes, s_elems, ncolh)
    in_maps = [{"slots": bufs[cc], "wts": wts, "b": b_col} for cc in range(NCORES)]
    trace = os.environ.get("KERNEL_TRACE", "0") == "1"
    kw_ = {}
    if trace:
        import tempfile
        kw_ = dict(trace=True, tmpdir=tempfile.mkdtemp(prefix="kernel_trace_"))
    res = run_bass_kernel_spmd(nc, in_maps, core_ids=list(range(NCORES)), **kw_)
    last_exec_ns = res.exec_time_ns

    # ---- host-side unpermute --------------------------------------------
    outs = np.stack([res.results[cc]["out"] for cc in range(NCORES)]) \
        .astype(np.float32)  # (8, 128, ncolh)
    full = np.empty((nd, D), np.float32)
    real = dom_core >= 0
    rc = dom_core[real]
    rh = dom_half[real]
    rcol = dom_hcol[real]
    rows = rh[:, None] * D + np.arange(D)[None, :]
    full[real] = outs[rc[:, None], rows, rcol[:, None]]
    if n0:
        full[~real] = b
    return full
